# revision 42
# baseline (speedup 1.0000x reference)
"""Trainium2 Bass kernel for nn_CalculateSLayer (GNN message passing).

Math: t[i,j,k,:] = tanh(hW[i] + E[matrix[i,j,k]] + b), E = emb @ W[60:],
masked by mask; s_in sums over (j,k), s_out over (i,k).  t depends only on
(i, c=matrix[i,j,k]) so per row i there are only 50 distinct values
T[i,c,:].  With z = mask ? matrix+1 : 0 (computed host-side, shipped bf16):

  s_out[j,f] = sum_{i,c} T[i,c,f] * #{k: z[i,j,k]=c+1}    (PE matmuls)
  s_in[i,f]  = sum_c hist[i,c] * T[i,c,f],  hist[i,c] = #{(j,k): z=c+1}

Planes ([128 x 2048] bf16 images consumed by PE as moving operands) are
produced on three engines concurrently, each with a fused accumulate that
yields hist for free:
  * c in [0, ND):        one-hot planes on DVE tensor_scalar
  * c in [ND, ND+NG):    one-hot planes on GpSimd tensor_scalar
  * c in [CA0, 50):      sign planes sgn(z-c-0.5) on ACT; telescoped
    coefficients V/2 plus a ones-plane with weight T[49]/2; hist from
    adjacent differences of the accumulated sign sums.

T chunks are computed high-c first so the ACT coefficient stream can
start early; W is broadcast in the moving access pattern (two
accumulating matmuls per chunk) so only ~30KB of weights are DMA'd.
Rows are sharded 128 per core over 8 cores; s_out partials are summed on
the host (the unshard step of the row-sharded reduction).
"""
import os
import sys
import numpy as np

sys.path.insert(0, "/opt/trn_rl_repo")

N = 1024
H2 = 60
DEP = 10
F = 70          # DOUT
NT = 50         # edge types
NCORES = 8
P = 128         # rows per core
JK = 2 * N      # (j, k) free elements per row, k innermost
ND = 26         # one-hot planes on DVE: c in [0, ND)
NG = 0          # one-hot planes on GpSimd: c in [ND, ND+NG)
CA0 = ND + NG   # ACT sign planes cover c in [CA0, 50)
NA = NT - CA0   # number of ACT sign planes

_CACHE = {}


def _build_nc():
    from concourse import bacc, mybir
    from concourse import tile

    f32 = mybir.dt.float32
    bf16 = mybir.dt.bfloat16
    Alu = mybir.AluOpType
    ActF = mybir.ActivationFunctionType

    nc = bacc.Bacc("TRN2", target_bir_lowering=False, debug=False,
                   num_devices=NCORES)

    zb_d = nc.dram_tensor("zb", [P, JK], bf16, kind="ExternalInput")
    h60_d = nc.dram_tensor("h60", [H2, P], bf16, kind="ExternalInput")
    w60_d = nc.dram_tensor("w60", [H2, F], bf16, kind="ExternalInput")
    eb_d = nc.dram_tensor("eb", [1, NT * F], bf16, kind="ExternalInput")
    sbias_d = nc.dram_tensor("sbias", [P, NA], f32, kind="ExternalInput")

    sin_d = nc.dram_tensor("s_in_part", [P, F], f32, kind="ExternalOutput")
    soutT_d = nc.dram_tensor("s_outT_part", [F, JK], bf16,
                             kind="ExternalOutput")

    # T-matmul chunks (14 types -> one 2-bank PSUM tile), high c first
    chunks = [(0, 14), (14, 14), (28, 14), (42, 8)][::-1]

    with tile.TileContext(nc) as tc:
        with (
            tc.tile_pool(name="const", bufs=1) as cpool,
            tc.tile_pool(name="work", bufs=2) as wpool,
            tc.tile_pool(name="pdve", bufs=10) as pdve,
            tc.tile_pool(name="pact", bufs=10) as pact,
            tc.tile_pool(name="pgp", bufs=3) as pgp,
            tc.tile_pool(name="ps_so", bufs=1, space="PSUM") as ps_so,
            tc.tile_pool(name="ps_t", bufs=2, space="PSUM") as ps_t,
        ):
            # ---- inputs (zb is the long pole; issue it first) ----
            zb = cpool.tile([P, JK], bf16, tag="zb")
            h60 = cpool.tile([H2, P], bf16, tag="h60")
            w60 = cpool.tile([H2, F], bf16, tag="w60")
            eb = cpool.tile([1, NT * F], bf16, tag="eb")
            sbias = cpool.tile([P, NA], f32, tag="sbias")
            nc.sync.dma_start(out=zb[:], in_=zb_d[:])
            nc.scalar.dma_start(out=h60[:], in_=h60_d[:])
            nc.scalar.dma_start(out=w60[:], in_=w60_d[:])
            nc.scalar.dma_start(out=eb[:], in_=eb_d[:])
            nc.scalar.dma_start(out=sbias[:], in_=sbias_d[:])

            ones = cpool.tile([P, 512], bf16, tag="ones")
            nc.gpsimd.memset(ones[:], 1.0)
            ones1 = cpool.tile([1, P], bf16, tag="ones1")
            nc.gpsimd.memset(ones1[:], 1.0)
            halfv = cpool.tile([P, 1], f32, tag="halfv")
            nc.gpsimd.memset(halfv[:], 0.5)
            jkv = cpool.tile([P, 1], f32, tag="jkv")
            nc.gpsimd.memset(jkv[:], float(JK))

            def bcast1(v, n):
                return v[:].rearrange("p (o c) -> p o c", o=1) \
                           .broadcast_to([P, n, 1])

            def gp_mul_bcast(out, in0, v):
                n = in0.shape[1]
                nc.gpsimd.tensor_tensor(
                    out=out.rearrange("p (a o) -> p a o", o=1),
                    in0=in0.rearrange("p (a o) -> p a o", o=1),
                    in1=bcast1(v, n), op=Alu.mult)

            # ---- T[i, c, f] = tanh(hW + E_c + b), chunks of 7 types.
            #      W is broadcast over c in the moving AP; the (E+b) row
            #      is added via a K=1 accumulating matmul. ----
            T_sb = cpool.tile([P, NT * F], bf16, tag="T")

            def t_tanh(c0, cnt, t_ps):
                h = cnt // 2
                t_v = t_ps[:].rearrange("p (s x) -> p s x", s=2)[:, :, 0:h * F]
                nc.scalar.activation(
                    out=T_sb[:, c0 * F:(c0 + cnt) * F].rearrange(
                        "p (s x) -> p s x", s=2),
                    in_=t_v, func=ActF.Tanh)

            held_tanh = None
            for c0, cnt in chunks:
                # two bank-aligned sub-chunks of h types each; one tanh
                # reads both via a strided view
                h = cnt // 2
                t_ps = ps_t.tile([P, 1024], f32, tag="tps", name=f"t_ps{c0}")
                w_b = w60[:].rearrange("k (o f) -> k o f", o=1) \
                            .broadcast_to([H2, h, F])
                for s in range(2):
                    nc.tensor.matmul(
                        out=t_ps[:, s * 512:s * 512 + h * F],
                        lhsT=h60[:], rhs=w_b,
                        start=True, stop=False)
                    nc.tensor.matmul(
                        out=t_ps[:, s * 512:s * 512 + h * F],
                        lhsT=ones1[:],
                        rhs=eb[:, (c0 + s * h) * F:(c0 + (s + 1) * h) * F],
                        start=False, stop=True)
                if c0 == 0:
                    # types 0..13 are needed only by DVE-plane consumes;
                    # defer this tanh so it doesn't stall the early ACT
                    # sign stream behind cold T-matmuls
                    held_tanh = (c0, cnt, t_ps)
                else:
                    t_tanh(c0, cnt, t_ps)

            # ---- s_out PSUM and helpers ----
            hist = cpool.tile([P, NT], f32, tag="hist")
            rpm = cpool.tile([P, NA], f32, tag="rpm")
            so_ps = ps_so.tile([F, JK], f32, tag="so")

            state = {"first": True}

            def consume(plane, wtile, woff, last=False):
                first = state["first"]
                state["first"] = False
                reuse = plane.shape[1] == 512
                for q in range(4):
                    nc.tensor.matmul(
                        out=so_ps[:, q * 512:(q + 1) * 512],
                        lhsT=wtile[:, woff:woff + F],
                        rhs=plane[:, 0:512] if reuse
                        else plane[:, q * 512:(q + 1) * 512],
                        start=first, stop=last)

            def dve_plane(c):
                mc = pdve.tile([P, JK], bf16, tag="mc", name=f"mc{c}")
                nc.vector.tensor_scalar(
                    out=mc[:], in0=zb[:], scalar1=float(c + 1),
                    scalar2=None, op0=Alu.is_equal, op1=Alu.add,
                    accum_out=hist[:, c:c + 1])
                consume(mc, T_sb, c * F)

            def gp_plane(c):
                mg = pgp.tile([P, JK], bf16, tag="mg", name=f"mg{c}")
                nc.gpsimd.tensor_scalar(
                    out=mg[:], in0=zb[:], scalar1=float(c + 1),
                    scalar2=None, op0=Alu.is_equal, op1=Alu.add,
                    accum_out=hist[:, c:c + 1])
                consume(mg, T_sb, c * F)

            # V2 coefficients for ACT planes, built in two pieces so the
            # first (high-r) sign planes can start before all T chunks
            # are done.  V2[r] = (T[CA0+r]-T[CA0+r-1])/2 for r>=1,
            # V2[0] = T[CA0]/2; ones-plane weight V2h = T[49]/2.
            V2 = cpool.tile([P, NA * F], bf16, tag="V2")

            def v2_piece(r_lo, r_hi):
                # entries r in [max(r_lo,1), r_hi)
                r0 = max(r_lo, 1)
                if r_hi > r0:
                    dm = wpool.tile([P, (NA - 1) * F], bf16, tag="dm",
                                    name=f"dm{r0}")
                    nc.gpsimd.tensor_tensor(
                        out=dm[:, (r0 - 1) * F:(r_hi - 1) * F],
                        in0=T_sb[:, (CA0 + r0) * F:(CA0 + r_hi) * F],
                        in1=T_sb[:, (CA0 + r0 - 1) * F:(CA0 + r_hi - 1) * F],
                        op=Alu.subtract)
                    gp_mul_bcast(V2[:, r0 * F:r_hi * F],
                                 dm[:, (r0 - 1) * F:(r_hi - 1) * F], halfv)
                if r_lo == 0:
                    gp_mul_bcast(V2[:, 0:F], T_sb[:, CA0 * F:(CA0 + 1) * F],
                                 halfv)

            def act_plane(r):
                sp = pact.tile([P, JK], bf16, tag="sp", name=f"sp{r}")
                nc.scalar.activation(
                    out=sp[:], in_=zb[:], func=ActF.Sign,
                    bias=sbias[:, r:r + 1],
                    accum_out=rpm[:, r:r + 1])
                consume(sp, V2, r * F)

            # ones-plane weight
            V2hb = cpool.tile([P, F], bf16, tag="V2hb")
            gp_mul_bcast(V2hb[:], T_sb[:, (NT - 1) * F:NT * F], halfv)

            # V2 high half first (T chunks arrive high-c first)
            RMID = NA // 2
            v2_piece(RMID, NA)

            # ---- zipped plane stream across ACT (desc r), DVE, GpSimd.
            #      Front-load DVE/GP slightly so the tail is ACT-only and
            #      the DVE epilogue piece for c<CA0 can run early. ----
            order = []
            na, nd, ng = NA, ND, NG
            ia = NA - 1
            id_, ig = ND - 1, ND
            tot = na + nd + ng
            ca = cd = cg = 0
            for s in range(tot):
                # pick stream with largest remaining fraction
                fa = (na - ca) / na if na else -1
                fd = (nd - cd) / nd * 1.22 if nd else -1
                fg = (ng - cg) / ng * 1.22 if ng else -1
                if fd >= fa and fd >= fg:
                    order.append(("d", id_)); id_ -= 1; cd += 1
                elif fg >= fa:
                    order.append(("g", ig)); ig += 1; cg += 1
                else:
                    order.append(("a", ia)); ia -= 1; ca += 1
            def sin_prod(c_lo, c_hi, tag):
                # contiguous [p, c, f] product on GpSimd (strided views
                # are slow there); the DVE reduce takes the strided
                # [p, f, c] view for free at 1x.
                ncnt = c_hi - c_lo
                t_cf = T_sb[:, c_lo * F:c_hi * F].rearrange(
                    "p (c f) -> p c f", c=ncnt)
                h_cf = hist[:, c_lo:c_hi].rearrange(
                    "p (c o) -> p c o", o=1).broadcast_to([P, ncnt, F])
                pr = wpool.tile([P, ncnt * F], f32, tag=f"pr_{tag}")
                nc.gpsimd.tensor_tensor(
                    out=pr[:].rearrange("p (c f) -> p c f", c=ncnt),
                    in0=t_cf, in1=h_cf, op=Alu.mult)
                return pr, ncnt

            def sin_reduce(pr, ncnt, tag):
                sr = wpool.tile([P, F], f32, tag=f"sr_{tag}")
                nc.vector.tensor_reduce(
                    out=sr[:],
                    in_=pr[:].rearrange("p (c f) -> p f c", c=ncnt),
                    axis=mybir.AxisListType.X, op=Alu.add)
                return sr

            def hist_hi_piece():
                # hist for c in [CA0+RMID, 50): needs rpm[RMID:NA] only
                # (the first NA-RMID sign planes, emitted descending).
                hd_hi = cpool.tile([P, NA - 1 - RMID], f32, tag="hd_hi")
                nc.gpsimd.tensor_tensor(
                    out=hd_hi[:], in0=rpm[:, RMID:NA - 1],
                    in1=rpm[:, RMID + 1:NA], op=Alu.subtract)
                gp_mul_bcast(hist[:, CA0 + RMID:NT - 1], hd_hi[:], halfv)
                hj = cpool.tile([P, 1], f32, tag="hj")
                nc.gpsimd.tensor_tensor(
                    out=hj[:], in0=rpm[:, NA - 1:NA],
                    in1=jkv[:], op=Alu.add)
                gp_mul_bcast(hist[:, NT - 1:NT], hj[:], halfv)

            emitted_v2lo = False
            emitted_mid = False
            pr_mid = None
            for kind, arg in order:
                if kind == "a":
                    if arg < RMID and not emitted_v2lo:
                        v2_piece(0, RMID)
                        emitted_v2lo = True
                    act_plane(arg)
                    if arg == RMID and not emitted_mid:
                        hist_hi_piece()
                        pr_mid = sin_prod(CA0 + RMID, NT, "mid")
                        emitted_mid = True
                elif kind == "d":
                    dve_plane(arg)
                else:
                    gp_plane(arg)
                zc = locals().setdefault("_zc", 0)
                if held_tanh is not None and sum(
                        1 for _ in order[:order.index((kind, arg)) + 1]) >= 4:
                    t_tanh(*held_tanh)
                    held_tanh = None
            # ---- T[i, c, f] = tanh(hW + E_c + b), chunks of 7 types.
            # ones plane: constant, 512-wide tile consumed 4x, closes PSUM
            consume(ones, V2hb, 0, last=True)

            # ---- s_out partial: evacuate PSUM on the (idle) ACT engine
            #      as a bf16 copy; the k-fold happens on the host along
            #      with the cross-core partial sum ----
            so_sb = wpool.tile([F, JK], bf16, tag="so_sb")
            nc.scalar.copy(out=so_sb[:], in_=so_ps[:])
            nc.scalar.dma_start(out=soutT_d[:], in_=so_sb[:])

            # s_in pieces: lo covers c in [0, CA0) (DVE hist complete)
            pr_lo = sin_prod(0, CA0, "lo")
            sin_mid = sin_reduce(*pr_mid, "mid")
            sin_lo = sin_reduce(*pr_lo, "lo")

            # ---- remaining hist (c in [CA0, CA0+RMID)) + last s_in piece
            hd_lo = cpool.tile([P, RMID], f32, tag="hd_lo")
            nc.gpsimd.tensor_tensor(
                out=hd_lo[:], in0=rpm[:, 0:RMID],
                in1=rpm[:, 1:RMID + 1], op=Alu.subtract)
            gp_mul_bcast(hist[:, CA0:CA0 + RMID], hd_lo[:], halfv)
            sin_tl = sin_reduce(*sin_prod(CA0, CA0 + RMID, "tl"), "tl")

            sin_a = wpool.tile([P, F], f32, tag="sin_a")
            nc.vector.tensor_tensor(
                out=sin_a[:], in0=sin_lo[:], in1=sin_mid[:], op=Alu.add)
            sin_sb = wpool.tile([P, F], f32, tag="sin_sb")
            nc.vector.tensor_tensor(
                out=sin_sb[:], in0=sin_a[:], in1=sin_tl[:], op=Alu.add)
            nc.sync.dma_start(out=sin_d[:], in_=sin_sb[:])

    nc.finalize()
    return nc


def _get_nc():
    if "nc" not in _CACHE:
        _CACHE["nc"] = _build_nc()
    return _CACHE["nc"]


def kernel(h, emb_table, W, b, matrix, mask):
    import ml_dtypes
    from concourse.bass_utils import run_bass_kernel_spmd

    bf16 = ml_dtypes.bfloat16
    h = np.asarray(h, dtype=np.float32)
    emb_table = np.asarray(emb_table, dtype=np.float32)
    W = np.asarray(W, dtype=np.float32)
    b = np.asarray(b, dtype=np.float32)
    matrix = np.asarray(matrix, dtype=np.int32)
    mask = np.asarray(mask, dtype=np.int32)

    # z = (matrix+1)*mask in {0 (dead), 1..50 (type c=z-1)}; exact in bf16
    z = ((matrix + 1) * mask).astype(bf16)

    E = emb_table @ W[H2:]                       # [NT, F]
    eb = (E + b).reshape(1, NT * F).astype(bf16)
    w60 = np.ascontiguousarray(W[:H2]).astype(bf16)   # [60, 70]

    sbias = np.empty((P, NA), np.float32)
    for r in range(NA):
        sbias[:, r] = -(float(CA0 + r) + 0.5)

    in_maps = []
    for s in range(NCORES):
        rows = slice(s * P, (s + 1) * P)
        in_maps.append({
            "zb": np.ascontiguousarray(z[rows].reshape(P, JK)),
            "h60": np.ascontiguousarray(h[rows].T).astype(bf16),
            "w60": w60,
            "eb": eb,
            "sbias": sbias,
        })

    nc = _get_nc()
    trace = bool(int(os.environ.get("KERNEL_TRACE", "0")))
    if trace:
        try:
            import ntff_shim
            ntff_shim.install()
        except Exception:
            trace = False
    res = run_bass_kernel_spmd(nc, in_maps, core_ids=list(range(NCORES)),
                               trace=trace)
    _CACHE["last_exec_ns"] = res.exec_time_ns

    s_in = np.concatenate(
        [res.results[s]["s_in_part"] for s in range(NCORES)], axis=0)
    s_out = np.sum(
        [res.results[s]["s_outT_part"].astype(np.float32)
         .reshape(F, N, 2).sum(2) for s in range(NCORES)], axis=0).T
    return (np.ascontiguousarray(s_in),
            np.ascontiguousarray(s_out.astype(np.float32)))


# revision 43
# speedup vs baseline: 1.0438x; 1.0438x over previous
"""Trainium2 Bass kernel for nn_CalculateSLayer (GNN message passing).

Math: t[i,j,k,:] = tanh(hW[i] + E[matrix[i,j,k]] + b), E = emb @ W[60:],
masked by mask; s_in sums over (j,k), s_out over (i,k).  t depends only on
(i, c=matrix[i,j,k]) so per row i there are only 50 distinct values
T[i,c,:].  With z = mask ? matrix+1 : 0 (computed host-side, shipped bf16):

  s_out[j,f] = sum_{i,c} T[i,c,f] * #{k: z[i,j,k]=c+1}    (PE matmuls)
  s_in[i,f]  = sum_c hist[i,c] * T[i,c,f],  hist[i,c] = #{(j,k): z=c+1}

Planes ([128 x 2048] bf16 images consumed by PE as moving operands) are
produced on three engines concurrently, each with a fused accumulate that
yields hist for free:
  * c in [0, ND):        one-hot planes on DVE tensor_scalar
  * c in [ND, ND+NG):    one-hot planes on GpSimd tensor_scalar
  * c in [CA0, 50):      sign planes sgn(z-c-0.5) on ACT; telescoped
    coefficients V/2 plus a ones-plane with weight T[49]/2; hist from
    adjacent differences of the accumulated sign sums.

T chunks are computed high-c first so the ACT coefficient stream can
start early; W is broadcast in the moving access pattern (two
accumulating matmuls per chunk) so only ~30KB of weights are DMA'd.
Rows are sharded 128 per core over 8 cores; s_out partials are summed on
the host (the unshard step of the row-sharded reduction).
"""
import os
import sys
import numpy as np

sys.path.insert(0, "/opt/trn_rl_repo")

N = 1024
H2 = 60
DEP = 10
F = 70          # DOUT
NT = 50         # edge types
NCORES = 8
P = 128         # rows per core
JK = 2 * N      # (j, k) free elements per row, k innermost
ND = 25         # one-hot planes on DVE: c in [0, ND)
NG = 0          # one-hot planes on GpSimd: c in [ND, ND+NG)
CA0 = ND + NG   # ACT sign planes cover c in [CA0, 50)
NA = NT - CA0   # number of ACT sign planes

_CACHE = {}


def _build_nc():
    from concourse import bacc, mybir
    from concourse import tile

    f32 = mybir.dt.float32
    bf16 = mybir.dt.bfloat16
    Alu = mybir.AluOpType
    ActF = mybir.ActivationFunctionType

    nc = bacc.Bacc("TRN2", target_bir_lowering=False, debug=False,
                   num_devices=NCORES)

    zb_d = nc.dram_tensor("zb", [P, JK], bf16, kind="ExternalInput")
    h60_d = nc.dram_tensor("h60", [H2, P], bf16, kind="ExternalInput")
    w60_d = nc.dram_tensor("w60", [H2, F], bf16, kind="ExternalInput")
    eb_d = nc.dram_tensor("eb", [1, NT * F], bf16, kind="ExternalInput")
    sbias_d = nc.dram_tensor("sbias", [P, NA], f32, kind="ExternalInput")

    sin_d = nc.dram_tensor("s_in_part", [P, F], f32, kind="ExternalOutput")
    soutT_d = nc.dram_tensor("s_outT_part", [F, JK], bf16,
                             kind="ExternalOutput")

    # T-matmul chunks (14 types -> one 2-bank PSUM tile), high c first
    chunks = [(0, 14), (14, 14), (28, 14), (42, 8)][::-1]

    with tile.TileContext(nc) as tc:
        with (
            tc.tile_pool(name="const", bufs=1) as cpool,
            tc.tile_pool(name="work", bufs=2) as wpool,
            tc.tile_pool(name="pdve", bufs=10) as pdve,
            tc.tile_pool(name="pact", bufs=10) as pact,
            tc.tile_pool(name="pgp", bufs=3) as pgp,
            tc.tile_pool(name="ps_so", bufs=1, space="PSUM") as ps_so,
            tc.tile_pool(name="ps_t", bufs=2, space="PSUM") as ps_t,
        ):
            # ---- inputs (zb is the long pole; issue it first) ----
            zb = cpool.tile([P, JK], bf16, tag="zb")
            h60 = cpool.tile([H2, P], bf16, tag="h60")
            w60 = cpool.tile([H2, F], bf16, tag="w60")
            eb = cpool.tile([1, NT * F], bf16, tag="eb")
            sbias = cpool.tile([P, NA], f32, tag="sbias")
            nc.sync.dma_start(out=zb[:], in_=zb_d[:])
            nc.scalar.dma_start(out=h60[:], in_=h60_d[:])
            nc.scalar.dma_start(out=w60[:], in_=w60_d[:])
            nc.scalar.dma_start(out=eb[:], in_=eb_d[:])
            nc.scalar.dma_start(out=sbias[:], in_=sbias_d[:])

            ones = cpool.tile([P, 512], bf16, tag="ones")
            nc.gpsimd.memset(ones[:], 1.0)
            ones1 = cpool.tile([1, P], bf16, tag="ones1")
            nc.gpsimd.memset(ones1[:], 1.0)
            halfv = cpool.tile([P, 1], f32, tag="halfv")
            nc.gpsimd.memset(halfv[:], 0.5)
            jkv = cpool.tile([P, 1], f32, tag="jkv")
            nc.gpsimd.memset(jkv[:], float(JK))

            def bcast1(v, n):
                return v[:].rearrange("p (o c) -> p o c", o=1) \
                           .broadcast_to([P, n, 1])

            def gp_mul_bcast(out, in0, v):
                n = in0.shape[1]
                nc.gpsimd.tensor_tensor(
                    out=out.rearrange("p (a o) -> p a o", o=1),
                    in0=in0.rearrange("p (a o) -> p a o", o=1),
                    in1=bcast1(v, n), op=Alu.mult)

            # ---- T[i, c, f] = tanh(hW + E_c + b), chunks of 7 types.
            #      W is broadcast over c in the moving AP; the (E+b) row
            #      is added via a K=1 accumulating matmul. ----
            T_sb = cpool.tile([P, NT * F], bf16, tag="T")

            def t_tanh(c0, cnt, t_ps):
                h = cnt // 2
                t_v = t_ps[:].rearrange("p (s x) -> p s x", s=2)[:, :, 0:h * F]
                nc.scalar.activation(
                    out=T_sb[:, c0 * F:(c0 + cnt) * F].rearrange(
                        "p (s x) -> p s x", s=2),
                    in_=t_v, func=ActF.Tanh)

            held_tanh = None
            for c0, cnt in chunks:
                # two bank-aligned sub-chunks of h types each; one tanh
                # reads both via a strided view
                h = cnt // 2
                t_ps = ps_t.tile([P, 1024], f32, tag="tps", name=f"t_ps{c0}")
                w_b = w60[:].rearrange("k (o f) -> k o f", o=1) \
                            .broadcast_to([H2, h, F])
                for s in range(2):
                    nc.tensor.matmul(
                        out=t_ps[:, s * 512:s * 512 + h * F],
                        lhsT=h60[:], rhs=w_b,
                        start=True, stop=False)
                    nc.tensor.matmul(
                        out=t_ps[:, s * 512:s * 512 + h * F],
                        lhsT=ones1[:],
                        rhs=eb[:, (c0 + s * h) * F:(c0 + (s + 1) * h) * F],
                        start=False, stop=True)
                if c0 == 0:
                    # types 0..13 are needed only by DVE-plane consumes;
                    # defer this tanh so it doesn't stall the early ACT
                    # sign stream behind cold T-matmuls
                    held_tanh = (c0, cnt, t_ps)
                else:
                    t_tanh(c0, cnt, t_ps)

            # ---- s_out PSUM and helpers ----
            hist = cpool.tile([P, NT], f32, tag="hist")
            rpm = cpool.tile([P, NA], f32, tag="rpm")
            so_ps = ps_so.tile([F, JK], f32, tag="so")

            state = {"first": True}

            def consume(plane, wtile, woff, last=False):
                first = state["first"]
                state["first"] = False
                reuse = plane.shape[1] == 512
                for q in range(4):
                    nc.tensor.matmul(
                        out=so_ps[:, q * 512:(q + 1) * 512],
                        lhsT=wtile[:, woff:woff + F],
                        rhs=plane[:, 0:512] if reuse
                        else plane[:, q * 512:(q + 1) * 512],
                        start=first, stop=last)

            def dve_plane(c):
                mc = pdve.tile([P, JK], bf16, tag="mc", name=f"mc{c}")
                nc.vector.tensor_scalar(
                    out=mc[:], in0=zb[:], scalar1=float(c + 1),
                    scalar2=None, op0=Alu.is_equal, op1=Alu.add,
                    accum_out=hist[:, c:c + 1])
                consume(mc, T_sb, c * F)

            def gp_plane(c):
                mg = pgp.tile([P, JK], bf16, tag="mg", name=f"mg{c}")
                nc.gpsimd.tensor_scalar(
                    out=mg[:], in0=zb[:], scalar1=float(c + 1),
                    scalar2=None, op0=Alu.is_equal, op1=Alu.add,
                    accum_out=hist[:, c:c + 1])
                consume(mg, T_sb, c * F)

            # V2 coefficients for ACT planes, built in two pieces so the
            # first (high-r) sign planes can start before all T chunks
            # are done.  V2[r] = (T[CA0+r]-T[CA0+r-1])/2 for r>=1,
            # V2[0] = T[CA0]/2; ones-plane weight V2h = T[49]/2.
            V2 = cpool.tile([P, NA * F], bf16, tag="V2")

            def v2_piece(r_lo, r_hi):
                # entries r in [max(r_lo,1), r_hi)
                r0 = max(r_lo, 1)
                if r_hi > r0:
                    dm = wpool.tile([P, (NA - 1) * F], bf16, tag="dm",
                                    name=f"dm{r0}")
                    nc.gpsimd.tensor_tensor(
                        out=dm[:, (r0 - 1) * F:(r_hi - 1) * F],
                        in0=T_sb[:, (CA0 + r0) * F:(CA0 + r_hi) * F],
                        in1=T_sb[:, (CA0 + r0 - 1) * F:(CA0 + r_hi - 1) * F],
                        op=Alu.subtract)
                    gp_mul_bcast(V2[:, r0 * F:r_hi * F],
                                 dm[:, (r0 - 1) * F:(r_hi - 1) * F], halfv)
                if r_lo == 0:
                    gp_mul_bcast(V2[:, 0:F], T_sb[:, CA0 * F:(CA0 + 1) * F],
                                 halfv)

            def act_plane(r):
                sp = pact.tile([P, JK], bf16, tag="sp", name=f"sp{r}")
                nc.scalar.activation(
                    out=sp[:], in_=zb[:], func=ActF.Sign,
                    bias=sbias[:, r:r + 1],
                    accum_out=rpm[:, r:r + 1])
                consume(sp, V2, r * F)

            # ones-plane weight
            V2hb = cpool.tile([P, F], bf16, tag="V2hb")
            gp_mul_bcast(V2hb[:], T_sb[:, (NT - 1) * F:NT * F], halfv)

            # V2 high half first (T chunks arrive high-c first)
            RMID = NA // 2
            v2_piece(RMID, NA)

            # ---- zipped plane stream across ACT (desc r), DVE, GpSimd.
            #      Front-load DVE/GP slightly so the tail is ACT-only and
            #      the DVE epilogue piece for c<CA0 can run early. ----
            order = []
            na, nd, ng = NA, ND, NG
            ia = NA - 1
            id_, ig = ND - 1, ND
            tot = na + nd + ng
            ca = cd = cg = 0
            for s in range(tot):
                # pick stream with largest remaining fraction
                fa = (na - ca) / na if na else -1
                fd = (nd - cd) / nd * 1.22 if nd else -1
                fg = (ng - cg) / ng * 1.22 if ng else -1
                if fd >= fa and fd >= fg:
                    order.append(("d", id_)); id_ -= 1; cd += 1
                elif fg >= fa:
                    order.append(("g", ig)); ig += 1; cg += 1
                else:
                    order.append(("a", ia)); ia -= 1; ca += 1
            def sin_prod(c_lo, c_hi, tag):
                # contiguous [p, c, f] product on GpSimd (strided views
                # are slow there); the DVE reduce takes the strided
                # [p, f, c] view for free at 1x.
                ncnt = c_hi - c_lo
                t_cf = T_sb[:, c_lo * F:c_hi * F].rearrange(
                    "p (c f) -> p c f", c=ncnt)
                h_cf = hist[:, c_lo:c_hi].rearrange(
                    "p (c o) -> p c o", o=1).broadcast_to([P, ncnt, F])
                pr = wpool.tile([P, ncnt * F], f32, tag=f"pr_{tag}")
                nc.gpsimd.tensor_tensor(
                    out=pr[:].rearrange("p (c f) -> p c f", c=ncnt),
                    in0=t_cf, in1=h_cf, op=Alu.mult)
                return pr, ncnt

            def sin_reduce(pr, ncnt, tag):
                sr = wpool.tile([P, F], f32, tag=f"sr_{tag}")
                nc.vector.tensor_reduce(
                    out=sr[:],
                    in_=pr[:].rearrange("p (c f) -> p f c", c=ncnt),
                    axis=mybir.AxisListType.X, op=Alu.add)
                return sr

            def hist_hi_piece():
                # hist for c in [CA0+RMID, 50): needs rpm[RMID:NA] only
                # (the first NA-RMID sign planes, emitted descending).
                hd_hi = cpool.tile([P, NA - 1 - RMID], f32, tag="hd_hi")
                nc.gpsimd.tensor_tensor(
                    out=hd_hi[:], in0=rpm[:, RMID:NA - 1],
                    in1=rpm[:, RMID + 1:NA], op=Alu.subtract)
                gp_mul_bcast(hist[:, CA0 + RMID:NT - 1], hd_hi[:], halfv)
                hj = cpool.tile([P, 1], f32, tag="hj")
                nc.gpsimd.tensor_tensor(
                    out=hj[:], in0=rpm[:, NA - 1:NA],
                    in1=jkv[:], op=Alu.add)
                gp_mul_bcast(hist[:, NT - 1:NT], hj[:], halfv)

            emitted_v2lo = False
            emitted_mid = False
            pr_mid = None
            for kind, arg in order:
                if kind == "a":
                    if arg < RMID and not emitted_v2lo:
                        v2_piece(0, RMID)
                        emitted_v2lo = True
                    act_plane(arg)
                    if arg == RMID and not emitted_mid:
                        hist_hi_piece()
                        pr_mid = sin_prod(CA0 + RMID, NT, "mid")
                        emitted_mid = True
                elif kind == "d":
                    dve_plane(arg)
                else:
                    gp_plane(arg)
                zc = locals().setdefault("_zc", 0)
                if held_tanh is not None and sum(
                        1 for _ in order[:order.index((kind, arg)) + 1]) >= 4:
                    t_tanh(*held_tanh)
                    held_tanh = None
            # ---- T[i, c, f] = tanh(hW + E_c + b), chunks of 7 types.
            # ones plane: constant, 512-wide tile consumed 4x, closes PSUM
            consume(ones, V2hb, 0, last=True)

            # ---- s_out partial: evacuate PSUM on the (idle) ACT engine
            #      as a bf16 copy; the k-fold happens on the host along
            #      with the cross-core partial sum ----
            so_sb = wpool.tile([F, JK], bf16, tag="so_sb")
            nc.scalar.copy(out=so_sb[:], in_=so_ps[:])
            nc.scalar.dma_start(out=soutT_d[:], in_=so_sb[:])

            # s_in pieces: lo covers c in [0, CA0) (DVE hist complete)
            pr_lo = sin_prod(0, CA0, "lo")
            sin_mid = sin_reduce(*pr_mid, "mid")
            sin_lo = sin_reduce(*pr_lo, "lo")

            # ---- remaining hist (c in [CA0, CA0+RMID)) + last s_in piece
            hd_lo = cpool.tile([P, RMID], f32, tag="hd_lo")
            nc.gpsimd.tensor_tensor(
                out=hd_lo[:], in0=rpm[:, 0:RMID],
                in1=rpm[:, 1:RMID + 1], op=Alu.subtract)
            gp_mul_bcast(hist[:, CA0:CA0 + RMID], hd_lo[:], halfv)
            sin_tl = sin_reduce(*sin_prod(CA0, CA0 + RMID, "tl"), "tl")

            sin_a = wpool.tile([P, F], f32, tag="sin_a")
            nc.vector.tensor_tensor(
                out=sin_a[:], in0=sin_lo[:], in1=sin_mid[:], op=Alu.add)
            sin_sb = wpool.tile([P, F], f32, tag="sin_sb")
            nc.vector.tensor_tensor(
                out=sin_sb[:], in0=sin_a[:], in1=sin_tl[:], op=Alu.add)
            nc.sync.dma_start(out=sin_d[:], in_=sin_sb[:])

    nc.finalize()
    return nc


def _get_nc():
    if "nc" not in _CACHE:
        _CACHE["nc"] = _build_nc()
    return _CACHE["nc"]


def kernel(h, emb_table, W, b, matrix, mask):
    import ml_dtypes
    from concourse.bass_utils import run_bass_kernel_spmd

    bf16 = ml_dtypes.bfloat16
    h = np.asarray(h, dtype=np.float32)
    emb_table = np.asarray(emb_table, dtype=np.float32)
    W = np.asarray(W, dtype=np.float32)
    b = np.asarray(b, dtype=np.float32)
    matrix = np.asarray(matrix, dtype=np.int32)
    mask = np.asarray(mask, dtype=np.int32)

    # z = (matrix+1)*mask in {0 (dead), 1..50 (type c=z-1)}; exact in bf16
    z = ((matrix + 1) * mask).astype(bf16)

    E = emb_table @ W[H2:]                       # [NT, F]
    eb = (E + b).reshape(1, NT * F).astype(bf16)
    w60 = np.ascontiguousarray(W[:H2]).astype(bf16)   # [60, 70]

    sbias = np.empty((P, NA), np.float32)
    for r in range(NA):
        sbias[:, r] = -(float(CA0 + r) + 0.5)

    in_maps = []
    for s in range(NCORES):
        rows = slice(s * P, (s + 1) * P)
        in_maps.append({
            "zb": np.ascontiguousarray(z[rows].reshape(P, JK)),
            "h60": np.ascontiguousarray(h[rows].T).astype(bf16),
            "w60": w60,
            "eb": eb,
            "sbias": sbias,
        })

    nc = _get_nc()
    trace = bool(int(os.environ.get("KERNEL_TRACE", "0")))
    if trace:
        try:
            import ntff_shim
            ntff_shim.install()
        except Exception:
            trace = False
    res = run_bass_kernel_spmd(nc, in_maps, core_ids=list(range(NCORES)),
                               trace=trace)
    _CACHE["last_exec_ns"] = res.exec_time_ns

    s_in = np.concatenate(
        [res.results[s]["s_in_part"] for s in range(NCORES)], axis=0)
    s_out = np.sum(
        [res.results[s]["s_outT_part"].astype(np.float32)
         .reshape(F, N, 2).sum(2) for s in range(NCORES)], axis=0).T
    return (np.ascontiguousarray(s_in),
            np.ascontiguousarray(s_out.astype(np.float32)))
